# revision 33
# baseline (speedup 1.0000x reference)
"""Chebyshev approximation kernel for Trainium2 (8 NeuronCores, SPMD data-parallel).

Math: reference computes
    y_at_nodes = (1-t) * y[:, idx] + t * y[:, idx+1]      # [n_obs, deg]
    out        = (y_at_nodes @ basis).reshape(-1)         # [n_obs*deg]
Factorized device kernel: out = (y @ W) @ B where W [2049, 1024] holds the
two interp weights per node column (banded, since idx is monotone) and B is
the dense basis. B is a DCT-II matrix: basis[deg-1-j, k] = (-1)^k
basis[j, k], so with g = u_j + u_{deg-1-j}, h = u_j - u_{deg-1-j} (j <
deg/2) the even output columns need only g @ Bg and the odd columns h @ Bh,
each a 512-contraction GEMM - half the FLOPs of the dense u @ B. The fold
is free on the PE: GEMM1 emits psum pairs (A_i from W columns of j-tile i,
D_i from mirrored j-tile 7-i with host-reversed columns so partitions
align) and the PSUM->SBUF drains become DVE add/subs (D staged through
SBUF by the ACT engine because the DVE reads at most one PSUM operand).
Even/odd outputs interleave through a [128, 512, 2] SBUF view.

y is cast to bf16 ON HOST and trimmed to a 16x128 k-tile grid (the lone
t*y[:, 2048] interp term that spills past column 2048 is applied on host
as an exact fp32 rank-1 update), so the device does plain bf16 loads
(half the HBM read traffic of fp32, no cast pass, no memsets),
PE-transposes each block (bf16 transpose = 1 cycle/row), runs banded
GEMM1 on m=512 groups, and GEMM2 against the folded half-basis. Output
stored bf16 (halves store DMA), upcast on host.
Per-group PE order gemm1(g) -> transposes(g+1) -> gemm2(g) keeps the PE
>90% busy by giving the DVE a transpose-slot to drain the g/h folds.

Sharding: y rows split 8192/core across 8 cores; W/Bg/Bh replicated. The
band structure (not the W values) is baked at compile time and cached by
its signature, so recompiles only happen if x changes shape qualitatively.
"""

import os
import numpy as np

DEG = 1024
N_OBS = 65536
M_P1 = 2049
N_CORES = 8
ROWS_PER_CORE = N_OBS // N_CORES  # 8192
RB = 128                          # rows per block
GB = 4                            # blocks per GEMM1 group (m = 512)
KT = 16                           # k tiles of 128 covering m < 2048
KP = KT * 128                     # 2048 (the y[:, 2048] term is a host-side
                                  # rank-1 correction, see _prep/kernel)
JT = 8                            # node j-tiles (1024/128)
JH = 4                            # folded half: 512 = 4 tiles

_COMPILED = {}
_PREP_CACHE = {}
LAST_RESULTS = None


def _prep(x: np.ndarray):
    """Host precompute: paired banded W (bf16), folded Bg/Bh (bf16), bands."""
    import ml_dtypes

    key = x.tobytes()
    hit = _PREP_CACHE.get(key)
    if hit is not None:
        return hit
    x = np.asarray(x, dtype=np.float32)
    k = np.arange(DEG, dtype=np.float32)
    ang = (np.float32(np.pi) * (k + np.float32(0.5))) / np.float32(DEG)
    nodes = np.sort(np.cos(ang.astype(np.float32)).astype(np.float32))
    idx = np.clip(np.searchsorted(x, nodes, side="right") - 1, 0, M_P1 - 2)
    a = x[idx]
    b = x[idx + 1]
    t = ((nodes - a) / (b - a)).astype(np.float64)
    W = np.zeros((KP, DEG), dtype=np.float64)
    W[idx, np.arange(DEG)] += 1.0 - t
    hi = idx + 1
    spill = hi >= KP  # terms touching y[:, 2048]: applied on host (rank-1)
    W[hi[~spill], np.arange(DEG)[~spill]] += t[~spill]

    norm = ((np.float32(2.0) - (k == 0).astype(np.float32)) / np.float32(DEG)).astype(
        np.float64
    )
    theta = np.arccos(nodes.astype(np.float64))
    basis = norm[None, :] * np.cos(k.astype(np.float64)[None, :] * theta[:, None])

    # band: per j-tile, the k-tiles containing any nonzero of W (rows are
    # clamped to the 16-tile grid; the spilled row is the host fix)
    bands = []
    for jt in range(JT):
        lo = int(idx[jt * 128 : (jt + 1) * 128].min()) // 128
        hi_r = min(int(idx[jt * 128 : (jt + 1) * 128].max()) + 1, KP - 1) // 128
        bands.append(tuple(range(lo, hi_r + 1)))
    bands = tuple(bands)

    # pack W band tiles pair-major: for mirror pair i: A tiles (columns of
    # j-tile i), then D tiles (columns of j-tile 7-i, column-reversed so
    # D psum partition p holds u[:, deg-1-(i*128+p)]). One DMA total.
    nband = sum(len(bd) for bd in bands)
    W_pk = np.empty((128, nband * 128), dtype=np.float64)
    s = 0
    for i in range(JH):
        for kt in bands[i]:
            W_pk[:, s * 128 : (s + 1) * 128] = W[
                kt * 128 : (kt + 1) * 128, i * 128 : (i + 1) * 128
            ]
            s += 1
        for kt in bands[JT - 1 - i]:
            W_pk[:, s * 128 : (s + 1) * 128] = W[
                kt * 128 : (kt + 1) * 128,
                (JT - 1 - i) * 128 : (JT - i) * 128,
            ][:, ::-1]
            s += 1
    W_bf = np.ascontiguousarray(W_pk.astype(ml_dtypes.bfloat16))

    # folded basis halves: even cols from the symmetric part, odd from the
    # antisymmetric part (exact up to the ~1e-6 float32 node asymmetry).
    Bg = (basis[: DEG // 2, 0::2] + basis[DEG - 1 : DEG // 2 - 1 : -1, 0::2]) / 2
    Bh = (basis[: DEG // 2, 1::2] - basis[DEG - 1 : DEG // 2 - 1 : -1, 1::2]) / 2
    Bg_pk = Bg.reshape(JH, 128, 512).transpose(1, 0, 2).reshape(128, JH * 512)
    Bh_pk = Bh.reshape(JH, 128, 512).transpose(1, 0, 2).reshape(128, JH * 512)
    B_bf = np.ascontiguousarray(
        np.concatenate([Bg_pk, Bh_pk], axis=1).astype(ml_dtypes.bfloat16)
    )
    # host-side correction: u[:, j] += t_j * y[:, 2048] for spilled terms,
    # i.e. out += outer(t_j * y[:, 2048], basis[j, :])
    fixes = [
        (float(t[j]), basis[j, :].astype(np.float32))
        for j in np.nonzero(spill)[0]
    ]
    out = (W_bf, B_bf, bands, fixes)
    _PREP_CACHE[key] = out
    return out


def build_cheb_kernel(tc, y_ap, w_ap, b_ap, id_ap, o_ap, rows, bands):
    import concourse.mybir as mybir

    nc = tc.nc
    f32 = mybir.dt.float32
    bf16 = mybir.dt.bfloat16
    add_op = mybir.AluOpType.add
    sub_op = mybir.AluOpType.subtract
    nb = rows // RB
    ngrp = nb // GB

    with (
        tc.tile_pool(name="consts", bufs=1) as consts,
        tc.tile_pool(name="ycpool", bufs=9) as ycpool,
        tc.tile_pool(name="ytg", bufs=3) as ytgpool,
        tc.tile_pool(name="ynt", bufs=2) as yntpool,
        tc.tile_pool(name="dpool", bufs=3) as dpool,
        tc.tile_pool(name="opool", bufs=3) as opool,
        tc.tile_pool(name="pst", bufs=2, space="PSUM") as pstp,
        tc.tile_pool(name="p1", bufs=3, space="PSUM") as p1p,
        tc.tile_pool(name="pso", bufs=3, space="PSUM") as psop,
    ):
        ident = consts.tile([128, 128], bf16)
        nc.scalar.dma_start(out=ident, in_=id_ap)
        nband = sum(len(bd) for bd in bands)
        b_sb = consts.tile([128, 2 * JH * 512], bf16)
        w_sb = consts.tile([128, nband * 128], bf16)

        # slot order mirrors the host pack: pair i -> A band tiles, D band
        # tiles (D weights already column-reversed host-side).
        slot = {}
        s = 0
        for i in range(JH):
            for kt in bands[i]:
                slot[("A", i, kt)] = s
                s += 1
            for kt in bands[JT - 1 - i]:
                slot[("D", i, kt)] = s
                s += 1

        def load_w():
            # host-packed partition-major: one dma_start (needed by gemm1(0))
            nc.scalar.dma_start(out=w_sb, in_=w_ap)

        def load_b():
            # 1MB basis: deferred past the first-group y loads (needed only
            # by gemm2(0), ~8us later) to keep startup HBM for y0..y3 + W
            nc.scalar.dma_start(out=b_sb, in_=b_ap)

        ycs, ytgs, ghs = {}, {}, {}

        def load_y(b):
            # y is bf16 + zero-padded to 2176 cols host-side: one plain
            # HWDGE DMA per block, no cast pass, no memset.
            yc = ycpool.tile([128, KP], bf16, name="yc", tag="yc")
            nc.sync.dma_start(out=yc, in_=y_ap[b * RB : (b + 1) * RB, :])
            ycs[b] = yc

        def trans_block(b):
            g = b % GB
            if g == 0:
                ytgs[b // GB] = ytgpool.tile(
                    [128, KT, GB * 128], bf16, name="ytg", tag="ytg"
                )
            ytg = ytgs[b // GB]
            yc = ycs[b]
            for half in range(2):  # 8 k-tiles per pst tile, one wide copy
                pst = pstp.tile([128, 8, 128], bf16, name="pst", tag="pst")
                k0 = half * 8
                for ji in range(8):
                    kt = k0 + ji
                    nc.tensor.transpose(
                        pst[:, ji, :], yc[:, kt * 128 : (kt + 1) * 128], ident
                    )
                dst = ytg[:, k0 : k0 + 8, g * 128 : (g + 1) * 128]
                if half == 0:
                    nc.vector.tensor_copy(dst, pst)
                else:
                    nc.scalar.copy(dst, pst)
            del ycs[b]

        def gemm1(grp):
            # psum pair per mirror pair i: A_i (j-tile i), D_i (mirrored
            # j-tile, partition-aligned); g/h tiles via DVE add/sub.
            ytg = ytgs[grp]
            gh = yntpool.tile([128, JT, GB * 128], bf16, name="gh", tag="gh")
            ghs[grp] = gh
            for i in range(JH):
                pd = p1p.tile([128, GB * 128], f32, name="pd", tag="p1")
                bdm = bands[JT - 1 - i]
                for n_, kt in enumerate(bdm):
                    sD = slot[("D", i, kt)]
                    nc.tensor.matmul(
                        pd,
                        w_sb[:, sD * 128 : (sD + 1) * 128],
                        ytg[:, kt, :],
                        start=(n_ == 0),
                        stop=(n_ == len(bdm) - 1),
                    )
                # DVE may read only one PSUM operand per op: stage D in SBUF
                # (ACT copy), then A +/- D with A still in PSUM.
                dsb = dpool.tile([128, GB * 128], f32, name="dsb", tag="dsb")
                nc.scalar.copy(dsb, pd)
                pa = p1p.tile([128, GB * 128], f32, name="pa", tag="p1")
                bd = bands[i]
                for n_, kt in enumerate(bd):
                    sA = slot[("A", i, kt)]
                    nc.tensor.matmul(
                        pa,
                        w_sb[:, sA * 128 : (sA + 1) * 128],
                        ytg[:, kt, :],
                        start=(n_ == 0),
                        stop=(n_ == len(bd) - 1),
                    )
                nc.vector.tensor_tensor(gh[:, i, :], pa, dsb, add_op)
                nc.vector.tensor_tensor(gh[:, JH + i, :], pa, dsb, sub_op)
            del ytgs[grp]

        def gemm2(b):
            g = b % GB
            gs = slice(g * 128, (g + 1) * 128)
            gh = ghs[b // GB]
            osb = opool.tile([128, 512, 2], bf16, name="osb", tag="osb")
            pe = psop.tile([128, 512], f32, name="pe", tag="ps")
            for i in range(JH):
                nc.tensor.matmul(
                    pe,
                    gh[:, i, gs],
                    b_sb[:, i * 512 : (i + 1) * 512],
                    start=(i == 0),
                    stop=(i == JH - 1),
                )
            nc.vector.tensor_copy(osb[:, :, 0], pe)
            po = psop.tile([128, 512], f32, name="po", tag="ps")
            for i in range(JH):
                nc.tensor.matmul(
                    po,
                    gh[:, JH + i, gs],
                    b_sb[:, (JH + i) * 512 : (JH + i + 1) * 512],
                    start=(i == 0),
                    stop=(i == JH - 1),
                )
            if b == nb - 1:
                # tail trim: drain the last psum on both engines and store
                # in halves so the store DMA overlaps the second drain
                nc.vector.tensor_copy(osb[:, 0:256, 1], po[:, 0:256])
                nc.scalar.copy(osb[:, 256:512, 1], po[:, 256:512])
                nc.scalar.dma_start(
                    out=o_ap[b * RB : (b + 1) * RB, 0:512], in_=osb[:, 0:256, :]
                )
                nc.scalar.dma_start(
                    out=o_ap[b * RB : (b + 1) * RB, 512:1024], in_=osb[:, 256:512, :]
                )
            else:
                nc.scalar.copy(osb[:, :, 1], po)
                nc.scalar.dma_start(out=o_ap[b * RB : (b + 1) * RB, :], in_=osb)
            if g == GB - 1:
                del ghs[b // GB]

        # prologue: first-group y loads beat the constant loads onto the
        # queues; W tiles land before gemm1(0), B before gemm2(0).
        load_y(0)
        load_w()
        for b in range(1, min(GB, nb)):
            load_y(b)
        for b in range(min(GB, nb)):
            trans_block(b)
        for b in range(GB, min(2 * GB, nb)):
            load_y(b)
        load_b()

        # PE order per group: gemm1(g), transposes for g+1 (giving DVE time
        # to finish g's gh add/subs), then gemm2(g). Group g+2 loads issue
        # after gemm1(g) so startup HBM goes to the data needed first.
        for grp in range(ngrp):
            gemm1(grp)
            for b in range((grp + 2) * GB, min((grp + 3) * GB, nb)):
                load_y(b)
            for b in range((grp + 1) * GB, min((grp + 2) * GB, nb)):
                trans_block(b)
            for b in range(grp * GB, (grp + 1) * GB):
                gemm2(b)


def _build_nc(rows, bands):
    import concourse.mybir as mybir
    import concourse.tile as tile
    from concourse import bacc

    f32 = mybir.dt.float32
    bf16 = mybir.dt.bfloat16
    nc = bacc.Bacc(
        "TRN2",
        target_bir_lowering=False,
        debug=False,
        enable_asserts=False,
        num_devices=N_CORES,
    )
    nband = sum(len(bd) for bd in bands)
    y_ap = nc.dram_tensor("y", [rows, KP], bf16, kind="ExternalInput").ap()
    w_ap = nc.dram_tensor("wmat", [128, nband * 128], bf16, kind="ExternalInput").ap()
    b_ap = nc.dram_tensor("bmat", [128, 2 * JH * 512], bf16, kind="ExternalInput").ap()
    id_ap = nc.dram_tensor("ident", [128, 128], bf16, kind="ExternalInput").ap()
    o_ap = nc.dram_tensor("o", [rows, DEG], bf16, kind="ExternalOutput").ap()
    with tile.TileContext(nc) as tc:
        build_cheb_kernel(tc, y_ap, w_ap, b_ap, id_ap, o_ap, rows, bands)
    nc.compile()
    return nc


def _get_compiled(rows, bands):
    key = (rows, bands)
    if key not in _COMPILED:
        _COMPILED[key] = _build_nc(rows, bands)
    return _COMPILED[key]


def kernel(x: np.ndarray, y: np.ndarray) -> np.ndarray:
    global LAST_RESULTS
    import ml_dtypes
    from concourse import bass_utils

    x = np.asarray(x, dtype=np.float32)
    y = np.asarray(y)
    assert y.shape == (N_OBS, M_P1), y.shape
    W_bf, B_bf, bands, fixes = _prep(x)

    nc = _get_compiled(ROWS_PER_CORE, bands)
    # bf16 on host: halves HBM read traffic; the last grid column (2048)
    # is handled by the host-side rank-1 fix, so the device sees exactly
    # the 16x128 k-tile grid.
    y_bf = np.ascontiguousarray(y[:, :KP].astype(ml_dtypes.bfloat16))
    ident = np.ascontiguousarray(np.eye(128, dtype=ml_dtypes.bfloat16))
    in_maps = [
        {
            "y": y_bf[i * ROWS_PER_CORE : (i + 1) * ROWS_PER_CORE],
            "wmat": W_bf,
            "bmat": B_bf,
            "ident": ident,
        }
        for i in range(N_CORES)
    ]
    trace = bool(int(os.environ.get("CHEB_TRACE", "0")))
    res = bass_utils.run_bass_kernel_spmd(
        nc, in_maps, core_ids=list(range(N_CORES)), trace=trace
    )
    LAST_RESULTS = res
    out = np.concatenate(
        [
            np.asarray(res.results[i]["o"]).astype(np.float32)
            for i in range(N_CORES)
        ],
        axis=0,
    )
    if fixes:
        ylast = np.asarray(y[:, KP], dtype=np.float32)
        for tj, brow in fixes:
            out += np.outer(tj * ylast, brow)
    return out.reshape(-1)
